# revision 2
# baseline (speedup 1.0000x reference)
"""Trainium2 Bass kernel for the leaky-ReLU arccos covariance-grid conv1d problem.

Computation (see problem reference):
  k: (B,B,N,T,2) f32.  k_gp = k[...,0], k_ntk = k[...,1]
  v[b,t] = k_gp[b,b,0,t];  std = sqrt(max(v,0)) padded with N-1 zeros
  std_x[b0,t] = std[b0,t];  std_y[b1,n,t] = std[b1,n+t]
  rho = clip(k_gp / max(std_x*std_y, EPS), +-RHO_LIM)
  With leak a (graded a=1): one_m=(1-a)^2=0, coef=1+a^2=2 =>
    c0 = std_x*std_y*rho  = min(k_gp, RHO_LIM*std_x*std_y)   (k_gp >= 0)
    c1 = 1
  kg = conv1d(c0, w, pad 1) + beta
  kn = conv1d(c0 + c1*k_ntk, w, pad 1) + beta     (conv is linear)
  out = stack([kg, kn], -1)

Sharding: b0 (leading batch axis) across 8 cores; each core handles the
(8,128,1024,2) slice k[b0] independently.  The tiny diagonal std table is
computed on host; the Hankel-expanded std_y table ships as fp16 (2 MiB/core).

Per-core device program per b1 tile (N=128 partitions, T=1024):
  Pool: M = sxm * sqh16 (fp16 upconvert in-op);  s = c0 + ntk
  DVE:  c0 = min(gp, M);  two 2-op ratio conv chains (kg over c0, kn over s)
  ACT:  final Copy(t2*w2 + beta) for both channels, interleaved writes
  DMA:  sync ring = loads (1 MiB x + 256 KiB sqh16 per b1), scalar = stores
"""

import numpy as np
from contextlib import ExitStack

import concourse.bass as bass
import concourse.tile as tile
from concourse import bacc, mybir
from concourse.alu_op_type import AluOpType
from concourse.bass_utils import run_bass_kernel_spmd

B, N, T = 8, 128, 1024
EPS = 1e-12
RHO_LIM = 1.0 - 1e-6
F32 = mybir.dt.float32
F16 = mybir.dt.float16

_prog_cache = {}


def _build_program(r0, r1, w2, wl, wc, wr, beta, use_ratio):
    """One SPMD program, identical on all 8 cores (data differs per core).

    Conv tap handling: if use_ratio, kg = ((xl*r0 + xc)*r1 + xr)*w2 with
    r0=w0/w1, r1=w1/w2 (2 DVE ops + scale folded into the ACT copy);
    otherwise the general 3-multiply form on DVE.
    """
    nc = bacc.Bacc(
        "TRN2",
        target_bir_lowering=False,
        debug=False,
        enable_asserts=False,
        num_devices=8,
    )
    x_d = nc.dram_tensor("x", [B, N, 2 * T], F32, kind="ExternalInput").ap()
    sqh_d = nc.dram_tensor("sqh", [B, N, T], F16, kind="ExternalInput").ap()
    sxm_d = nc.dram_tensor("sxm", [1, T], F32, kind="ExternalInput").ap()
    out_d = nc.dram_tensor("out", [B, N, 2 * T], F32, kind="ExternalOutput").ap()

    with tile.TileContext(nc) as tc, ExitStack() as ctx:
        const = ctx.enter_context(tc.tile_pool(name="const", bufs=1))
        inp_pool = ctx.enter_context(tc.tile_pool(name="inp", bufs=3))
        sqh_pool = ctx.enter_context(tc.tile_pool(name="sqh", bufs=3))
        out_pool = ctx.enter_context(tc.tile_pool(name="outp", bufs=3))
        m_pool = ctx.enter_context(tc.tile_pool(name="mp", bufs=2))
        t2_pool = ctx.enter_context(tc.tile_pool(name="t2p", bufs=2))
        t2n_pool = ctx.enter_context(tc.tile_pool(name="t2np", bufs=2))

        sxm_sb = const.tile([N, T], F32)
        sxr_sb = const.tile([1, T], F32)
        nc.sync.dma_start(sxr_sb[:], sxm_d)
        # broadcast the std_x row across partitions on the TensorEngine:
        # ones(1,128).T @ row(1,512-chunk) -> (128,512); exact for fp32
        ones_sb = const.tile([1, N], F32)
        nc.gpsimd.memset(ones_sb[:], 1.0)
        with tc.tile_pool(name="psx", bufs=1, space="PSUM") as psx_pool:
            psx = psx_pool.tile([N, T], F32, tag="psx")
            for chunk in range(T // 512):
                lo = chunk * 512
                nc.tensor.matmul(
                    psx[:, lo : lo + 512], ones_sb[:],
                    sxr_sb[:, lo : lo + 512],
                    start=True, stop=True,
                )
            nc.scalar.activation(
                sxm_sb[:], psx[:], mybir.ActivationFunctionType.Copy
            )

        # persistent padded work buffers (zeros at cols 0 and T+1 survive
        # across b1 iterations; only cols 1..T are rewritten)
        c0p = const.tile([N, T + 2], F32)
        sp = const.tile([N, T + 2], F32)
        t1_t = const.tile([N, T], F32)
        nc.vector.memset(c0p[:, 0:1], 0.0)
        nc.vector.memset(c0p[:, T + 1 : T + 2], 0.0)
        nc.gpsimd.memset(sp[:, 0:1], 0.0)
        nc.gpsimd.memset(sp[:, T + 1 : T + 2], 0.0)

        for b1 in range(B):
            inp = inp_pool.tile([N, 2 * T], F32, tag="inp")
            sqh = sqh_pool.tile([N, T], F16, tag="sqh")
            if b1 == 0:
                nc.sync.dma_start(sqh[:], sqh_d[b1])
                nc.sync.dma_start(inp[:, 0:T], x_d[b1, :, 0:T])
                nc.sync.dma_start(inp[:, T : 2 * T], x_d[b1, :, T : 2 * T])
            else:
                nc.sync.dma_start(sqh[:], sqh_d[b1])
                nc.sync.dma_start(inp[:], x_d[b1])
            # iv[:, t, c]: channel c value at time t
            iv = inp.rearrange("p (t c) -> p t c", c=2)

            # Pool: M = sxm * sqh16 (mixed fp16*f32 -> f32)
            m_t = m_pool.tile([N, T], F32, tag="m")
            nc.gpsimd.tensor_tensor(
                m_t[:], sqh[:], sxm_sb[:], op=AluOpType.mult
            )
            # DVE: c0 = min(gp, M)
            nc.vector.tensor_tensor(
                c0p[:, 1 : T + 1], iv[:, 0:T, 0], m_t[:], op=AluOpType.min
            )
            # Pool: s = c0 + ntk
            nc.gpsimd.tensor_tensor(
                sp[:, 1 : T + 1], c0p[:, 1 : T + 1], iv[:, 0:T, 1],
                op=AluOpType.add,
            )

            out = out_pool.tile([N, 2 * T], F32, tag="out")
            ov = out.rearrange("p (t c) -> p t c", c=2)
            t2_t = t2_pool.tile([N, T], F32, tag="t2")
            t2n_t = t2n_pool.tile([N, T], F32, tag="t2n")
            if use_ratio:
                # kg chain over c0 (padded buffer)
                nc.vector.scalar_tensor_tensor(
                    t1_t[:], c0p[:, 0:T], r0, c0p[:, 1 : T + 1],
                    AluOpType.mult, AluOpType.add,
                )
                nc.vector.scalar_tensor_tensor(
                    t2_t[:], t1_t[:], r1, c0p[:, 2 : T + 2],
                    AluOpType.mult, AluOpType.add,
                )
                nc.scalar.activation(
                    ov[:, :, 0], t2_t[:],
                    mybir.ActivationFunctionType.Copy, bias=beta, scale=w2,
                )
                # kn chain over s = c0 + ntk
                nc.vector.scalar_tensor_tensor(
                    t1_t[:], sp[:, 0:T], r0, sp[:, 1 : T + 1],
                    AluOpType.mult, AluOpType.add,
                )
                nc.vector.scalar_tensor_tensor(
                    t2n_t[:], t1_t[:], r1, sp[:, 2 : T + 2],
                    AluOpType.mult, AluOpType.add,
                )
                nc.scalar.activation(
                    ov[:, :, 1], t2n_t[:],
                    mybir.ActivationFunctionType.Copy, bias=beta, scale=w2,
                )
            else:
                # general taps: 3-multiply form on DVE, both channels
                for src, ch in ((c0p, 0), (sp, 1)):
                    nc.vector.tensor_scalar_mul(t1_t[:], src[:, 0:T], wl)
                    nc.vector.scalar_tensor_tensor(
                        t1_t[:], src[:, 1 : T + 1], wc, t1_t[:],
                        AluOpType.mult, AluOpType.add,
                    )
                    dst = t2_t if ch == 0 else t2n_t
                    nc.vector.scalar_tensor_tensor(
                        dst[:], src[:, 2 : T + 2], wr, t1_t[:],
                        AluOpType.mult, AluOpType.add,
                    )
                    nc.scalar.activation(
                        ov[:, :, ch], dst[:],
                        mybir.ActivationFunctionType.Copy, bias=beta, scale=1.0,
                    )
            eng = nc.sync if b1 + 1 == B else nc.scalar
            eng.dma_start(out_d[b1], out[:])

    nc.compile()
    return nc


def _host_reference(k, leak, alpha, beta):
    """Numpy fallback replicating the reference exactly (any leak/alpha)."""
    k_gp, k_ntk = k[..., 0], k[..., 1]
    Bb, _, Nn, Tt = k_gp.shape
    ar = np.arange(Bb)
    v = k_gp[ar, ar, 0, :]
    v_pad = np.pad(v, ((0, 0), (0, Nn - 1)))
    std = np.sqrt(np.maximum(v_pad, 0.0))
    std_x = std[:, :Tt][:, None, None, :]
    std_y = np.lib.stride_tricks.sliding_window_view(std, Tt, axis=1)[None]
    denom = np.maximum(std_x * std_y, EPS)
    rho = np.clip(k_gp / denom, -RHO_LIM, RHO_LIM).astype(np.float32)
    a = max(float(leak), 0.0)
    theta = np.arccos(rho)
    s = np.sqrt(1.0 - rho * rho)
    one_m = (1.0 - a) ** 2
    coef = 1.0 + a * a
    sxy = (std_x * std_y).astype(np.float32)
    c0 = sxy / (2 * np.pi) * (one_m * s + rho * (coef * np.pi - one_m * theta))
    c1 = (coef * np.pi - one_m * theta) / (2 * np.pi)
    w = np.maximum(np.asarray(alpha, np.float32).reshape(-1), 0.0)

    def conv(x):
        xp = np.pad(x, ((0, 0), (0, 0), (0, 0), (1, 1)))
        return (
            w[0] * xp[..., :Tt] + w[1] * xp[..., 1 : Tt + 1] + w[2] * xp[..., 2 : Tt + 2]
        ).astype(np.float32)

    b = max(float(beta), 0.0)
    kg = conv(c0.astype(np.float32)) + b
    kn = conv((c1 * k_ntk).astype(np.float32)) + (kg - b) + b
    return np.stack([kg, kn], axis=-1).astype(np.float32)


def kernel(k, leak, alpha, beta, _want_profile=False):
    k = np.ascontiguousarray(np.asarray(k, dtype=np.float32))
    a = max(float(np.asarray(leak)), 0.0)
    w = np.maximum(np.asarray(alpha, dtype=np.float32).reshape(-1), np.float32(0.0))
    b_eff = max(float(np.asarray(beta)), 0.0)

    fast = (a == 1.0) and k.min() >= 0.0 and w.shape[0] == 3
    if not fast:
        return _host_reference(k, leak, alpha, beta)

    wl, wc, wr = (float(x) for x in w)
    use_ratio = (wc != 0.0) and (wr != 0.0)
    r0 = float(np.float32(wl) / np.float32(wc)) if use_ratio else 0.0
    r1 = float(np.float32(wc) / np.float32(wr)) if use_ratio else 0.0

    key = (r0, r1, wl, wc, wr, b_eff, use_ratio)
    if key not in _prog_cache:
        _prog_cache[key] = _build_program(
            r0, r1, wr, wl, wc, wr, b_eff, use_ratio
        )
    nc = _prog_cache[key]

    # host-side tiny prep: diagonal std table (the sharding hint's "all-gather")
    ar = np.arange(B)
    v = k[ar, ar, 0, :, 0]                              # (B, T)
    v_pad = np.pad(v, ((0, 0), (0, N - 1)))             # (B, T+N-1)
    std16 = np.sqrt(np.maximum(v_pad, 0.0)).astype(np.float16)
    sqh16 = np.ascontiguousarray(
        np.lib.stride_tricks.sliding_window_view(std16, T, axis=1)
    )                                                   # (B, N, T) fp16: std[b, n+t]
    std32 = np.sqrt(np.maximum(v_pad[:, :T], 0.0)).astype(np.float32)

    rl = np.float32(RHO_LIM)
    in_maps = []
    for c in range(B):
        sxm = np.ascontiguousarray(rl * std32[c]).reshape(1, T).astype(np.float32)
        in_maps.append({
            "x": k[c].reshape(B, N, 2 * T),
            "sqh": sqh16,
            "sxm": sxm,
        })

    res = run_bass_kernel_spmd(
        nc, in_maps, core_ids=list(range(8)), trace=_want_profile
    )
    out = np.stack([r["out"].reshape(B, N, T, 2) for r in res.results], axis=0)
    if _want_profile:
        kernel.last_exec_time_ns = res.exec_time_ns
        kernel.last_results = res
    return out


kernel.last_exec_time_ns = None
kernel.last_results = None


# revision 5
# speedup vs baseline: 1.4432x; 1.4432x over previous
"""Trainium2 Bass kernel for the leaky-ReLU arccos covariance-grid conv1d problem.

Computation (see problem reference):
  k: (B,B,N,T,2) f32.  k_gp = k[...,0], k_ntk = k[...,1]
  v[b,t] = k_gp[b,b,0,t];  std = sqrt(max(v,0)) padded with N-1 zeros
  std_x[b0,t] = std[b0,t];  std_y[b1,n,t] = std[b1,n+t]
  rho = clip(k_gp / max(std_x*std_y, EPS), +-RHO_LIM)
  With leak a (graded a=1): one_m=(1-a)^2=0, coef=1+a^2=2 =>
    c0 = std_x*std_y*rho  = min(k_gp, RHO_LIM*std_x*std_y)   (k_gp >= 0)
    c1 = 1
  kg = conv1d(c0, w, pad 1) + beta
  kn = kg + conv1d(k_ntk, w, pad 1)          (c1 = 1)
  out = stack([kg, kn], -1)

Sharding: b0 (leading batch axis) across 8 cores; each core handles the
(8,128,1024,2) slice k[b0] independently.  The tiny diagonal std table is
computed on host; the Hankel-expanded std_y table ships as fp16 (2 MiB/core).

Engine split per b1 tile (N=128 partitions, T=1024), rel-err budget 2e-2:
  DVE:  M = sxm16*sqh16 (fp16, 2x mode); c0 = min(gp, M) -> fp16;
        kg ratio chain in fp16 (2 stt); combine kn = q*w0 + kg.
  ACT:  ntk cast f32->bf16; final Copy(t2*w2 + beta) interleaved write.
  PE:   ntk conv as 3 shifted bf16 identity matmuls accumulating in PSUM.
  DMA:  sync ring = loads (1 MiB x + 256 KiB sqh16 per b1), scalar = stores.
  Pool: memsets only (DVE/GpSimd share an exclusive SBUF port-pair lock, so
        Pool tensor ops would block DVE 2-port ops).
"""

import numpy as np
from contextlib import ExitStack

import concourse.bass as bass
import concourse.tile as tile
from concourse import bacc, mybir
from concourse.alu_op_type import AluOpType
from concourse.bass_utils import run_bass_kernel_spmd

B, N, T = 8, 128, 1024
EPS = 1e-12
RHO_LIM = 1.0 - 1e-6
F32 = mybir.dt.float32
F16 = mybir.dt.float16
BF16 = mybir.dt.bfloat16

_prog_cache = {}


def _build_program(r0, r1, w2, wl, wc, wr, beta, use_ratio, use_pe):
    """One SPMD program, identical on all 8 cores (data differs per core)."""
    nc = bacc.Bacc(
        "TRN2",
        target_bir_lowering=False,
        debug=False,
        enable_asserts=False,
        num_devices=8,
    )
    x_d = nc.dram_tensor("x", [B, N, 2 * T], F32, kind="ExternalInput").ap()
    sqh_d = nc.dram_tensor("sqh", [B, N, T], F16, kind="ExternalInput").ap()
    sxm_d = nc.dram_tensor("sxm", [1, T], F32, kind="ExternalInput").ap()
    if use_pe:
        id_d = nc.dram_tensor("ident", [N, N], BF16, kind="ExternalInput").ap()
    out_d = nc.dram_tensor("out", [B, N, 2 * T], F32, kind="ExternalOutput").ap()

    with tile.TileContext(nc) as tc, ExitStack() as ctx:
        const = ctx.enter_context(tc.tile_pool(name="const", bufs=1))
        inp_pool = ctx.enter_context(tc.tile_pool(name="inp", bufs=3))
        sqh_pool = ctx.enter_context(tc.tile_pool(name="sqh", bufs=3))
        out_pool = ctx.enter_context(tc.tile_pool(name="outp", bufs=3))
        t2_pool = ctx.enter_context(tc.tile_pool(name="t2p", bufs=2))
        if use_pe:
            ntk_pool = ctx.enter_context(tc.tile_pool(name="ntkp", bufs=2))

        sxm16 = const.tile([N, T], F16)
        sxr_sb = const.tile([1, T], F32)
        nc.sync.dma_start(sxr_sb[:], sxm_d)
        if use_pe:
            id_sb = const.tile([N, N], BF16)
            nc.scalar.dma_start(id_sb[:], id_d)
        # broadcast the std_x row across partitions on the TensorEngine:
        # ones(1,128).T @ row(1,512-chunk) -> (128,512); cast to fp16 on ACT
        ones_sb = const.tile([1, N], F32)
        nc.gpsimd.memset(ones_sb[:], 1.0)
        with tc.tile_pool(name="psx", bufs=1, space="PSUM") as psx_pool:
            psx = psx_pool.tile([N, T], F32, tag="psx")
            for chunk in range(T // 512):
                lo = chunk * 512
                nc.tensor.matmul(
                    psx[:, lo : lo + 512], ones_sb[:],
                    sxr_sb[:, lo : lo + 512],
                    start=True, stop=True,
                )
            nc.scalar.activation(
                sxm16[:], psx[:], mybir.ActivationFunctionType.Copy
            )
        if use_pe:
            psum_pool = ctx.enter_context(
                tc.tile_pool(name="psq", bufs=4, space="PSUM")
            )

        # persistent work tiles; padded-edge zeros survive b1 iterations
        m16 = const.tile([N, T], F16)
        c0p = const.tile([N, T + 2], F16)
        t1_t = const.tile([N, T], F16)
        nc.vector.memset(c0p[:, 0:1], 0.0)
        nc.vector.memset(c0p[:, T + 1 : T + 2], 0.0)

        for b1 in range(B):
            inp = inp_pool.tile([N, 2 * T], F32, tag="inp")
            sqh = sqh_pool.tile([N, T], F16, tag="sqh")
            if b1 == 0:
                nc.sync.dma_start(sqh[:], sqh_d[b1])
                nc.sync.dma_start(inp[:, 0:T], x_d[b1, :, 0:T])
                nc.sync.dma_start(inp[:, T : 2 * T], x_d[b1, :, T : 2 * T])
            else:
                nc.sync.dma_start(sqh[:], sqh_d[b1])
                nc.sync.dma_start(inp[:], x_d[b1])
            # iv[:, t, c]: channel c value at time t
            iv = inp.rearrange("p (t c) -> p t c", c=2)

            # DVE: M = sxm * sqh (fp16 2x); c0 = min(gp, M) -> fp16
            nc.vector.tensor_tensor(
                m16[:], sqh[:], sxm16[:], op=AluOpType.mult
            )
            nc.vector.tensor_tensor(
                c0p[:, 1 : T + 1], iv[:, 0:T, 0], m16[:], op=AluOpType.min
            )

            out = out_pool.tile([N, 2 * T], F32, tag="out")
            ov = out.rearrange("p (t c) -> p t c", c=2)
            t2_t = t2_pool.tile([N, T], F16, tag="t2")
            if use_pe:
                # ACT: cast ntk channel to bf16 padded tile for the PE conv
                ntk16 = ntk_pool.tile([N, T + 2], BF16, tag="ntk")
                nc.vector.memset(ntk16[:, 0:1], 0.0)
                nc.vector.memset(ntk16[:, T + 1 : T + 2], 0.0)
                nc.scalar.activation(
                    ntk16[:, 1 : T + 1], iv[:, 0:T, 1],
                    mybir.ActivationFunctionType.Copy,
                )
                # kg ratio chain on DVE (fp16 2x)
                nc.vector.scalar_tensor_tensor(
                    t1_t[:], c0p[:, 0:T], r0, c0p[:, 1 : T + 1],
                    AluOpType.mult, AluOpType.add,
                )
                nc.vector.scalar_tensor_tensor(
                    t2_t[:], t1_t[:], r1, c0p[:, 2 : T + 2],
                    AluOpType.mult, AluOpType.add,
                )
                nc.scalar.activation(
                    ov[:, :, 0], t2_t[:],
                    mybir.ActivationFunctionType.Copy, bias=beta, scale=w2,
                )
                # k_ntk conv on the TensorEngine: sum of 3 shifted channels
                q = psum_pool.tile([N, T], F32, tag="q")
                for chunk in range(T // 512):
                    lo = chunk * 512
                    for j in range(3):
                        nc.tensor.matmul(
                            q[:, lo : lo + 512],
                            id_sb[:],
                            ntk16[:, j + lo : j + lo + 512],
                            start=(j == 0),
                            stop=(j == 2),
                        )
                # kn = w0 * conv_sum(ntk) + kg  (taps equal => w0)
                nc.vector.scalar_tensor_tensor(
                    ov[:, :, 1], q[:], wl, ov[:, :, 0],
                    AluOpType.mult, AluOpType.add,
                )
            elif use_ratio:
                # fp16 s-chain: s = c0 + ntk; kn = conv(s) (linearity, c1=1)
                sp = const.tile([N, T + 2], F16, tag="spbuf")
                if b1 == 0:
                    nc.vector.memset(sp[:, 0:1], 0.0)
                    nc.vector.memset(sp[:, T + 1 : T + 2], 0.0)
                nc.vector.tensor_tensor(
                    sp[:, 1 : T + 1], iv[:, 0:T, 1], c0p[:, 1 : T + 1],
                    op=AluOpType.add,
                )
                nc.vector.scalar_tensor_tensor(
                    t1_t[:], c0p[:, 0:T], r0, c0p[:, 1 : T + 1],
                    AluOpType.mult, AluOpType.add,
                )
                nc.vector.scalar_tensor_tensor(
                    t2_t[:], t1_t[:], r1, c0p[:, 2 : T + 2],
                    AluOpType.mult, AluOpType.add,
                )
                nc.scalar.activation(
                    ov[:, :, 0], t2_t[:],
                    mybir.ActivationFunctionType.Copy, bias=beta, scale=w2,
                )
                t2n_t = t2_pool.tile([N, T], F16, tag="t2n")
                nc.vector.scalar_tensor_tensor(
                    t1_t[:], sp[:, 0:T], r0, sp[:, 1 : T + 1],
                    AluOpType.mult, AluOpType.add,
                )
                nc.vector.scalar_tensor_tensor(
                    t2n_t[:], t1_t[:], r1, sp[:, 2 : T + 2],
                    AluOpType.mult, AluOpType.add,
                )
                nc.scalar.activation(
                    ov[:, :, 1], t2n_t[:],
                    mybir.ActivationFunctionType.Copy, bias=beta, scale=w2,
                )
            else:
                # general taps: 3-multiply form on DVE, both channels
                sp = const.tile([N, T + 2], F16, tag="spbuf")
                if b1 == 0:
                    nc.vector.memset(sp[:, 0:1], 0.0)
                    nc.vector.memset(sp[:, T + 1 : T + 2], 0.0)
                nc.vector.tensor_tensor(
                    sp[:, 1 : T + 1], iv[:, 0:T, 1], c0p[:, 1 : T + 1],
                    op=AluOpType.add,
                )
                for src, ch in ((c0p, 0), (sp, 1)):
                    dst = t2_pool.tile([N, T], F16, tag=f"t2g{ch}")
                    nc.vector.tensor_scalar_mul(t1_t[:], src[:, 0:T], wl)
                    nc.vector.scalar_tensor_tensor(
                        t1_t[:], src[:, 1 : T + 1], wc, t1_t[:],
                        AluOpType.mult, AluOpType.add,
                    )
                    nc.vector.scalar_tensor_tensor(
                        dst[:], src[:, 2 : T + 2], wr, t1_t[:],
                        AluOpType.mult, AluOpType.add,
                    )
                    nc.scalar.activation(
                        ov[:, :, ch], dst[:],
                        mybir.ActivationFunctionType.Copy, bias=beta, scale=1.0,
                    )
            eng = nc.sync if b1 + 1 == B else nc.scalar
            eng.dma_start(out_d[b1], out[:])

    nc.compile()
    return nc


def _host_reference(k, leak, alpha, beta):
    """Numpy fallback replicating the reference exactly (any leak/alpha)."""
    k_gp, k_ntk = k[..., 0], k[..., 1]
    Bb, _, Nn, Tt = k_gp.shape
    ar = np.arange(Bb)
    v = k_gp[ar, ar, 0, :]
    v_pad = np.pad(v, ((0, 0), (0, Nn - 1)))
    std = np.sqrt(np.maximum(v_pad, 0.0))
    std_x = std[:, :Tt][:, None, None, :]
    std_y = np.lib.stride_tricks.sliding_window_view(std, Tt, axis=1)[None]
    denom = np.maximum(std_x * std_y, EPS)
    rho = np.clip(k_gp / denom, -RHO_LIM, RHO_LIM).astype(np.float32)
    a = max(float(leak), 0.0)
    theta = np.arccos(rho)
    s = np.sqrt(1.0 - rho * rho)
    one_m = (1.0 - a) ** 2
    coef = 1.0 + a * a
    sxy = (std_x * std_y).astype(np.float32)
    c0 = sxy / (2 * np.pi) * (one_m * s + rho * (coef * np.pi - one_m * theta))
    c1 = (coef * np.pi - one_m * theta) / (2 * np.pi)
    w = np.maximum(np.asarray(alpha, np.float32).reshape(-1), 0.0)

    def conv(x):
        xp = np.pad(x, ((0, 0), (0, 0), (0, 0), (1, 1)))
        return (
            w[0] * xp[..., :Tt] + w[1] * xp[..., 1 : Tt + 1] + w[2] * xp[..., 2 : Tt + 2]
        ).astype(np.float32)

    b = max(float(beta), 0.0)
    kg = conv(c0.astype(np.float32)) + b
    kn = conv((c1 * k_ntk).astype(np.float32)) + (kg - b) + b
    return np.stack([kg, kn], axis=-1).astype(np.float32)


def kernel(k, leak, alpha, beta, _want_profile=False):
    k = np.ascontiguousarray(np.asarray(k, dtype=np.float32))
    a = max(float(np.asarray(leak)), 0.0)
    w = np.maximum(np.asarray(alpha, dtype=np.float32).reshape(-1), np.float32(0.0))
    b_eff = max(float(np.asarray(beta)), 0.0)

    fast = (a == 1.0) and k.min() >= 0.0 and w.shape[0] == 3
    if not fast:
        return _host_reference(k, leak, alpha, beta)

    wl, wc, wr = (float(x) for x in w)
    use_ratio = (wc != 0.0) and (wr != 0.0)
    use_pe = use_ratio and (wl == wc == wr)
    r0 = float(np.float32(wl) / np.float32(wc)) if use_ratio else 0.0
    r1 = float(np.float32(wc) / np.float32(wr)) if use_ratio else 0.0

    key = (r0, r1, wl, wc, wr, b_eff, use_ratio, use_pe)
    if key not in _prog_cache:
        _prog_cache[key] = _build_program(
            r0, r1, wr, wl, wc, wr, b_eff, use_ratio, use_pe
        )
    nc = _prog_cache[key]

    # host-side tiny prep: diagonal std table (the sharding hint's "all-gather")
    ar = np.arange(B)
    v = k[ar, ar, 0, :, 0]                              # (B, T)
    v_pad = np.pad(v, ((0, 0), (0, N - 1)))             # (B, T+N-1)
    std16 = np.sqrt(np.maximum(v_pad, 0.0)).astype(np.float16)
    sqh16 = np.ascontiguousarray(
        np.lib.stride_tricks.sliding_window_view(std16, T, axis=1)
    )                                                   # (B, N, T) fp16: std[b, n+t]
    std32 = np.sqrt(np.maximum(v_pad[:, :T], 0.0)).astype(np.float32)

    rl = np.float32(RHO_LIM)
    if use_pe:
        import ml_dtypes

        ident = np.eye(N, dtype=ml_dtypes.bfloat16)
    in_maps = []
    for c in range(B):
        sxm = np.ascontiguousarray(rl * std32[c]).reshape(1, T).astype(np.float32)
        m = {
            "x": k[c].reshape(B, N, 2 * T),
            "sqh": sqh16,
            "sxm": sxm,
        }
        if use_pe:
            m["ident"] = ident
        in_maps.append(m)

    res = run_bass_kernel_spmd(
        nc, in_maps, core_ids=list(range(8)), trace=_want_profile
    )
    out = np.stack([r["out"].reshape(B, N, T, 2) for r in res.results], axis=0)
    if _want_profile:
        kernel.last_exec_time_ns = res.exec_time_ns
        kernel.last_results = res
    return out


kernel.last_exec_time_ns = None
kernel.last_results = None
